# revision 65
# baseline (speedup 1.0000x reference)
"""Trainium2 Bass kernel for nn_AgnosticResidualInteractionBlock (GNN message passing).

Strategy (8 NeuronCores, receiver-node graph partition; all constant scales and
the pre-message linear W_pre are folded into host-side weights):
  - Host: pack nodes into 16-receiver *windows*, each with EXACTLY 16 nodes and
    256 incoming edges (perfect bin-packing with swap repair; falls back to the
    greedy <=256 packer), so the per-core edge stream has zero padding;
    pre-gather raw sender features into a per-core bf16 edge stream; fold
    W_pre @ W_post into four combined 128x128 matrices so the device scatters
    RAW features and applies one output linear per block; pack all small
    constants into one DMA.
  - Device (SPMD, one Bass program on 8 cores), per 128-receiver tile:
      radial MLP -> per-edge mix scalars; one batched is_equal builds the
      16-chunk one-hot slab, one dual-broadcast multiply expands it into the
      8-block scatter rhs; six matmuls per chunk scatter-accumulate into PSUM
      (fg streamed in two half-tile DMAs aligned with the two PSUM banks);
      post-message linear with folded weights. The species linear (sc) is
      interleaved into iterations 2..NT-3 so its compute and stores hide under
      the edge stream. DMA rings are split: loads on SP (sync), nout stores on
      Act (scalar), sc stores on SWDGE (gpsimd) so stores never stall loads.
  - Host: map receiver slots back to node ids, cast outputs to f32.
"""
import sys
import numpy as np

try:
    import concourse.bacc as bacc
except ImportError:  # pragma: no cover
    sys.path.insert(0, "/opt/trn_rl_repo")
    import concourse.bacc as bacc

import ml_dtypes

import concourse.bass as bass
import concourse.mybir as mybir
import concourse.tile as tile
from concourse.bass_utils import run_bass_kernel_spmd

BF16 = ml_dtypes.bfloat16
N, C, E, S = 16384, 128, 262144, 10
NCORES = 8
AVG = 16.0
INV_C = 1.0 / np.sqrt(C)
INV2C = 1.0 / np.sqrt(2 * C)
INV_SQRT3 = 1.0 / np.sqrt(3.0)
W = 16                  # receivers per window
WCAP = 256              # edge slots per window (2 chunks)

_CACHE = {}


def _cpack_layout(NCH):
    """Column offsets of the packed small-constant tensor [128, CP]."""
    o = {}
    o["wm1"] = 0            # [8, 64] on partitions 0:8
    o["wm2"] = 64           # [128, 64]
    o["wm3"] = 128          # [128, 4]
    o["wcomb"] = 132        # [128, 512]
    o["iota"] = 644         # [128, 256]
    o["rloc"] = 900         # [128, NCH]
    o["aT"] = 900 + NCH     # [128, NCH*3]
    o["end"] = 900 + 4 * NCH
    return o


def _snake_repair(items_deg, groups, per, target, rng, iters=300000):
    """Partition `len(items_deg)` items into `groups` groups of exactly `per`
    items with degree-sum exactly `target` each. Returns idx [groups, per] of
    item indices, or None."""
    order = np.argsort(-items_deg, kind="stable")
    idx = np.empty((groups, per), np.int64)
    for r in range(per):
        chunk = order[r * groups:(r + 1) * groups]
        idx[:, r] = chunk if r % 2 == 0 else chunk[::-1]
    sums = items_deg[idx].sum(1)
    for _ in range(iters):
        dev = sums - target
        over = np.nonzero(dev > 0)[0]
        if len(over) == 0:
            return idx
        under = np.nonzero(dev < 0)[0]
        a = over[np.argmax(dev[over])]
        b = under[np.argmin(dev[under])]
        for attempt in range(2):
            da, db = items_deg[idx[a]], items_deg[idx[b]]
            diff = da[:, None] - db[None, :]
            cost = np.abs(dev[a] - diff) + np.abs(dev[b] + diff)
            i, j = np.unravel_index(np.argmin(cost), cost.shape)
            if cost[i, j] < abs(dev[a]) + abs(dev[b]):
                idx[a, i], idx[b, j] = idx[b, j], idx[a, i]
                sums[a] = items_deg[idx[a]].sum()
                sums[b] = items_deg[idx[b]].sum()
                break
            a = rng.choice(over)
            b = rng.choice(under)
        else:
            continue
    return None


def _balanced_core_assign(deg, node_specie):
    """Assign nodes to cores: per-species counts equal (up to +-1 padded to a
    common T[s]) AND per-core degree sums exactly E/NCORES. Returns
    (core_of, T) or None."""
    target = E // NCORES
    rng = np.random.default_rng(1)
    core_nodes = [[] for _ in range(NCORES)]
    counts = np.zeros(NCORES, np.int64)
    T = np.zeros(S, np.int64)
    for s in range(S):
        ns = np.nonzero(node_specie == s)[0]
        ns = ns[np.argsort(-deg[ns], kind="stable")]
        T[s] = -(-len(ns) // NCORES)
        q, r = divmod(len(ns), NCORES)
        # ceil-share goes to the cores with the lowest running totals so
        # every core ends with exactly N/NCORES nodes overall
        share = np.full(NCORES, q, np.int64)
        share[np.argsort(counts, kind="stable")[:r]] += 1
        pos = 0
        take = share.copy()
        i = 0
        while pos < len(ns):
            chunk = ns[pos:pos + NCORES]
            if i % 2 == 1:
                chunk = chunk[::-1]
            for n in chunk:
                k = int(np.argmax(take))
                core_nodes[k].append(n)
                take[k] -= 1
            pos += len(chunk)
            i += 1
        counts += share
    core_nodes = [np.array(c) for c in core_nodes]
    cdeg = np.array([deg[c].sum() for c in core_nodes])
    # repair with same-species swaps between cores
    for _ in range(200000):
        dev = cdeg - target
        over = np.nonzero(dev > 0)[0]
        if len(over) == 0:
            break
        under = np.nonzero(dev < 0)[0]
        a = over[np.argmax(dev[over])]
        b = under[np.argmin(dev[under])]
        best = None
        for s in range(S):
            ia = np.nonzero(node_specie[core_nodes[a]] == s)[0]
            ib = np.nonzero(node_specie[core_nodes[b]] == s)[0]
            if len(ia) == 0 or len(ib) == 0:
                continue
            da = deg[core_nodes[a][ia]]
            db = deg[core_nodes[b][ib]]
            diff = da[:, None] - db[None, :]
            cost = np.abs(dev[a] - diff) + np.abs(dev[b] + diff)
            i, j = np.unravel_index(np.argmin(cost), cost.shape)
            if best is None or cost[i, j] < best[0]:
                best = (cost[i, j], ia[i], ib[j])
        if best is None or best[0] >= abs(dev[a]) + abs(dev[b]):
            a = rng.choice(over)
            b = rng.choice(under)
            continue
        _, i, j = best
        na, nb = core_nodes[a][i], core_nodes[b][j]
        core_nodes[a][i], core_nodes[b][j] = nb, na
        cdeg[a] = deg[core_nodes[a]].sum()
        cdeg[b] = deg[core_nodes[b]].sum()
    else:
        pass
    if np.any(cdeg != target):
        return None
    core_of = np.zeros(N, np.int64)
    for k in range(NCORES):
        core_of[core_nodes[k]] = k
    return core_of, T


def _perfect_pack(deg, node_specie):
    """Pack nodes into NCORES*128 windows with EXACTLY 16 nodes and 256 edges
    each (zero padding), with per-core species counts balanced to a common
    vector T. Returns (core_of, win_of, slot_of, T) or None."""
    nwin = NCORES * 128
    if deg.sum() != nwin * WCAP or len(deg) != nwin * W or deg.max() > WCAP:
        return None
    res = _balanced_core_assign(deg, node_specie)
    if res is None:
        return None
    core_of, T = res
    win_of = np.zeros(N, np.int64)
    slot_of = np.zeros(N, np.int64)
    rng = np.random.default_rng(2)
    for k in range(NCORES):
        nk = np.nonzero(core_of == k)[0]
        idx = _snake_repair(deg[nk], 128, W, WCAP, rng)
        if idx is None:
            return None
        for w in range(128):
            for s in range(W):
                n = nk[idx[w, s]]
                win_of[n] = w
                slot_of[n] = s
    return core_of, win_of, slot_of, T


def _pack_windows(deg, nwc):
    """Greedy best-fit-decreasing fallback: nodes -> (core, window) with <=16
    nodes and <=256 edges per window. Returns tuple or None."""
    nwin = NCORES * nwc
    cap = np.full(nwin, WCAP, np.int64)
    slots = np.full(nwin, W, np.int64)
    core_of = np.zeros(N, np.int64)
    win_of = np.zeros(N, np.int64)
    slot_of = np.zeros(N, np.int64)
    order = np.argsort(-deg, kind="stable")
    eligible = np.ones(nwin, bool)
    for n in order:
        d = deg[n]
        c = np.where(eligible, cap, -1)
        w = int(np.argmax(c))
        if c[w] < d:
            return None
        cap[w] -= d
        slots[w] -= 1
        slot_of[n] = W - 1 - slots[w]
        if slots[w] == 0:
            eligible[w] = False
        core_of[n] = w // nwc
        win_of[n] = w % nwc
    return core_of, win_of, slot_of


def _host_prep(node_specie, node_feats, edge_attrs, edge_feats, senders, receivers,
               W_sc0, W_sc1, W_pre0, W_pre1, W_mlp1, W_mlp2, W_mlp3, W_post0, W_post1):
    senders = np.asarray(senders).astype(np.int64)
    receivers = np.asarray(receivers).astype(np.int64)
    node_specie = np.asarray(node_specie).astype(np.int64)
    node_feats = np.asarray(node_feats, dtype=np.float32)
    edge_attrs = np.asarray(edge_attrs, dtype=np.float32)
    edge_feats = np.asarray(edge_feats, dtype=np.float32)

    # ---- window packing (receiver partition balanced by degree) ------------
    deg = np.bincount(receivers, minlength=N)
    T = None
    res = _perfect_pack(deg, node_specie)
    if res is not None:
        core_of, win_of, slot_of, T = res
        nwc = 128
    else:
        nwc = 131
        while True:
            res = _pack_windows(deg, nwc)
            if res is not None:
                break
            nwc += 2
        core_of, win_of, slot_of = res
    NT = -(-nwc // 8)            # tiles per core
    NWC = NT * 8                 # padded windows per core
    NCH = NT * 16                # chunks per core
    EPADP = NCH * 128            # edge slots per core

    # ---- edge placement into per-core slot streams -------------------------
    er_core = core_of[receivers]
    er_win = win_of[receivers]
    gw = er_core * NWC + er_win
    eorder = np.argsort(gw, kind="stable")
    gw_s = gw[eorder]
    within = np.arange(E) - np.searchsorted(gw_s, gw_s)  # index within window
    slot_global = gw_s * WCAP + within                   # global slot id
    # per-core slot arrays
    send_slot = np.full(NCORES * EPADP, -1, np.int64)
    rloc_slot = np.full(NCORES * EPADP, -1.0, np.float32)
    a_slot = np.zeros((NCORES * EPADP, 3), np.float32)
    ef_slot = np.zeros((NCORES * EPADP, 8), np.float32)
    send_slot[slot_global] = senders[eorder]
    rloc_slot[slot_global] = slot_of[receivers[eorder]].astype(np.float32)
    a_slot[slot_global] = edge_attrs[eorder][:, 1:4]
    ef_slot[slot_global] = edge_feats[eorder]

    # ---- species permutation per core (sc stage) ---------------------------
    # Slot layout is a sequence of species runs at core-invariant offsets;
    # tiles may span species (the program emits one matmul set per run).
    nodes_of_core = [np.nonzero(core_of == k)[0] for k in range(NCORES)]
    if T is None:
        spec_counts = np.zeros((NCORES, S), np.int64)
        for k in range(NCORES):
            spec_counts[k] = np.bincount(node_specie[nodes_of_core[k]],
                                         minlength=S)
        T = np.maximum(1, -(-spec_counts.max(axis=0) // 128)) * 128
    run_off = np.concatenate([[0], np.cumsum(T)])
    NGT = -(-int(run_off[-1]) // 128)
    NSLOT = NGT * 128
    runs_of_tile = []
    for gt in range(NGT):
        lo_t, hi_t = gt * 128, (gt + 1) * 128
        rs = []
        for s in range(S):
            a, b = max(lo_t, run_off[s]), min(hi_t, run_off[s + 1])
            if b > a:
                rs.append((int(a - lo_t), int(b - a), int(s)))
        runs_of_tile.append(tuple(rs))
    runs_of_tile = tuple(runs_of_tile)
    node_of_scslot = -np.ones((NCORES, NSLOT), np.int64)   # global node ids
    for k in range(NCORES):
        nk = nodes_of_core[k]
        sp = node_specie[nk]
        for s in range(S):
            g = nk[sp == s]
            node_of_scslot[k, run_off[s] + np.arange(len(g))] = g

    # ---- weights (scales folded) -------------------------------------------
    sc_post = INV2C / AVG
    Wp0 = np.asarray(W_pre0, np.float64) * INV_C
    Wp1 = np.asarray(W_pre1, np.float64) * INV_C
    Wo0 = np.asarray(W_post0, np.float64) * sc_post
    Wo1 = np.asarray(W_post1, np.float64) * sc_post
    A = Wp0 @ Wo0[:, 0, :]
    B = (Wp1 @ Wo0[:, 1, :]) * INV_SQRT3
    Cm = Wp1 @ Wo1[:, 0, :]
    D = Wp0 @ Wo1[:, 1, :]
    wcomb = np.concatenate([A, B, Cm, D], axis=1).astype(np.float32)
    wsc0T = ((np.asarray(W_sc0, np.float32) * INV_C)
             .transpose(1, 0, 2).reshape(128, S * 128)).astype(BF16)
    wsc1T = ((np.asarray(W_sc1, np.float32) * INV_C)
             .transpose(1, 0, 2).reshape(128, S * 128)).astype(BF16)

    # ---- packed small constants -------------------------------------------
    co = _cpack_layout(NCH)
    cpack = np.zeros((128, co["end"]), np.float32)
    cpack[0:8, co["wm1"]:co["wm1"] + 64] = np.asarray(W_mlp1, np.float32) / np.sqrt(8.0)
    cpack[0:64, co["wm2"]:co["wm2"] + 64] = np.asarray(W_mlp2, np.float32) * 0.125
    cpack[64:128, co["wm2"]:co["wm2"] + 64] = cpack[0:64, co["wm2"]:co["wm2"] + 64]
    cpack[0:64, co["wm3"]:co["wm3"] + 4] = np.asarray(W_mlp3, np.float32) * 0.125
    cpack[64:128, co["wm3"]:co["wm3"] + 4] = cpack[0:64, co["wm3"]:co["wm3"] + 4]
    cpack[:, co["wcomb"]:co["wcomb"] + 512] = wcomb
    cpack[:, co["iota"]:co["iota"] + 256] = np.tile(
        np.arange(W, dtype=np.float32), (128, 16))
    cpack_c = cpack.astype(BF16)   # per-core copies get rloc/aT appended below

    # ---- per-core device arrays -------------------------------------------
    nfb2 = np.ascontiguousarray(
        node_feats.transpose(0, 2, 1).reshape(N, 512)).astype(BF16)
    nfb3 = np.concatenate([nfb2, np.zeros((1, 512), BF16)], axis=0)
    per_core = []
    for k in range(NCORES):
        sl = slice(k * EPADP, (k + 1) * EPADP)
        snd = send_slot[sl]
        rows = nfb3[snd]                       # [-1] -> zero row
        rows[snd < 0] = 0
        fgS = np.ascontiguousarray(
            rows.reshape(NCH, 128, 512).transpose(1, 0, 2).reshape(128, NCH * 512))
        cpk = cpack_c.copy()
        cpk[:, co["rloc"]:co["rloc"] + NCH] = (
            rloc_slot[sl].reshape(NCH, 128).T.astype(BF16))
        cpk[:, co["aT"]:co["aT"] + NCH * 3] = (
            a_slot[sl].reshape(NCH, 128, 3).transpose(1, 0, 2)
            .reshape(128, NCH * 3).astype(BF16))
        efT = np.ascontiguousarray(ef_slot[sl].T).astype(BF16)
        nfT = np.zeros((4, 128, NSLOT), np.float32)
        valid = node_of_scslot[k] >= 0
        nfT[:, :, valid] = node_feats[node_of_scslot[k][valid]].transpose(2, 1, 0)
        per_core.append(dict(fgS=fgS, cpk=cpk, efT=efT, nfT=nfT.astype(BF16)))

    shared = dict(wsc0T=wsc0T, wsc1T=wsc1T)
    meta = dict(NT=NT, NCH=NCH, NSLOT=NSLOT, runs_of_tile=runs_of_tile)
    unshard = dict(core_of=core_of, win_of=win_of, slot_of=slot_of,
                   node_of_scslot=node_of_scslot, NT=NT, NSLOT=NSLOT)
    return meta, per_core, shared, unshard


def _dep(later, earlier):
    tile.add_dep_helper(later.ins, earlier.ins, sync=False, reason="order")


def _build(meta):
    NT, NCH, NSLOT = meta["NT"], meta["NCH"], meta["NSLOT"]
    runs_of_tile = meta["runs_of_tile"]
    NGT = NSLOT // 128
    EPADP = NCH * 128
    f32, bf16 = mybir.dt.float32, mybir.dt.bfloat16
    co = _cpack_layout(NCH)

    nc = bacc.Bacc("TRN2", target_bir_lowering=False)
    fgS = nc.dram_tensor("fgS", [128, NCH * 512], bf16, kind="ExternalInput")
    cpk = nc.dram_tensor("cpk", [128, co["end"]], bf16, kind="ExternalInput")
    efT = nc.dram_tensor("efT", [8, EPADP], bf16, kind="ExternalInput")
    nfT = nc.dram_tensor("nfT", [4, 128, NSLOT], bf16, kind="ExternalInput")
    wsc0T = nc.dram_tensor("wsc0T", [128, S * 128], bf16, kind="ExternalInput")
    wsc1T = nc.dram_tensor("wsc1T", [128, S * 128], bf16, kind="ExternalInput")
    # transposed: row gt*128+out_ch, col node*4+comp; host untransposes
    sc_out = nc.dram_tensor("sc_out", [NGT * 128, 512], bf16,
                            kind="ExternalOutput")
    nout = nc.dram_tensor("nout", [NT * 128, 512], bf16, kind="ExternalOutput")

    # distribute the NGT sc tiles over main iterations 2..NT-3, singles
    # first and doubles late (compute slack grows once the prologue clears)
    sc_of_t = [[] for _ in range(NT)]
    lo = min(2, NT - 1)
    hi = max(lo + 1, NT - 2)            # exclusive
    slots = list(range(lo, hi))
    g = 0
    for t in slots:
        if g < NGT:
            sc_of_t[t].append(g)
            g += 1
    for t in reversed(slots):
        if g < NGT:
            sc_of_t[t].append(g)
            g += 1
    while g < NGT:                       # fallback if very few slots
        sc_of_t[hi - 1].append(g)
        g += 1

    with tile.TileContext(nc) as tc:
        with tc.tile_pool(name="cst", bufs=1) as cst, \
             tc.tile_pool(name="sa2", bufs=3) as sa2, \
             tc.tile_pool(name="fg", bufs=8) as fgp, \
             tc.tile_pool(name="mlp", bufs=2) as mlpp, \
             tc.tile_pool(name="qp", bufs=3) as qp, \
             tc.tile_pool(name="ohp", bufs=2) as ohp, \
             tc.tile_pool(name="ev", bufs=2) as evp, \
             tc.tile_pool(name="psm", bufs=2, space="PSUM") as psm, \
             tc.tile_pool(name="psagg", bufs=1, space="PSUM") as psagg, \
             tc.tile_pool(name="pso", bufs=2, space="PSUM") as pso:
            # ---- prologue loads (SP ring; all zero-dep) --------------------
            cpk_sb = cst.tile([128, co["end"]], bf16)
            nc.sync.dma_start(cpk_sb[:], cpk[:])
            ef_sb = cst.tile([8, EPADP], bf16)
            nc.sync.dma_start(ef_sb[:], efT[:])
            wm1_sb = cpk_sb[0:8, co["wm1"]:co["wm1"] + 64]
            wm2_sb = cpk_sb[:, co["wm2"]:co["wm2"] + 64]
            wm3_sb = cpk_sb[:, co["wm3"]:co["wm3"] + 4]
            wcomb_sb = cpk_sb[:, co["wcomb"]:co["wcomb"] + 512]
            iota_v = cpk_sb[:, co["iota"]:co["iota"] + 256].rearrange(
                "p (c s) -> p c s", s=16)
            rloc_sb = cpk_sb[:, co["rloc"]:co["rloc"] + NCH]
            aT_v = cpk_sb[:, co["aT"]:co["aT"] + NCH * 3].rearrange(
                "p (c k) -> p c k", k=3)

            wsc_loaded = False
            nfc = []

            # ---- main loop: one 128-receiver tile per iteration ------------
            for t in range(NT):
                e0 = t * 2048          # first edge slot of tile
                c0 = t * 16            # first chunk of tile
                # last tile streams in quarters so its scatter overlaps the
                # final transfers; earlier tiles in halves
                csz = 4 if t == NT - 1 else 8
                fg_of_j = []
                for h in range(16 // csz):
                    fh = fgp.tile([128, csz, 512], bf16, tag="fg")
                    nc.sync.dma_start(
                        fh[:].rearrange("p a b -> p (a b)"),
                        fgS[:, (c0 + csz * h) * 512:
                            (c0 + csz * (h + 1)) * 512])
                    fg_of_j += [(fh, jl) for jl in range(csz)]
                if t == 0:
                    # sc-stage loads ride behind the first fg tile
                    wsc0_sb = cst.tile([128, S * 128], bf16)
                    nc.sync.dma_start(wsc0_sb[:], wsc0T[:])
                    wsc1_sb = cst.tile([128, S * 128], bf16)
                    nc.sync.dma_start(wsc1_sb[:], wsc1T[:])
                    for comp in range(4):
                        t_ = cst.tile([128, NSLOT], bf16, name=f"nfc{comp}")
                        nc.sync.dma_start(t_[:], nfT[comp, :, :])
                        nfc.append(t_)

                # -- radial MLP for this tile's 2048 edges (2x1024 packed)
                ps_h = psm.tile([128, 1024], f32, tag="ps_h", space="PSUM")
                mm = []
                for half in range(2):
                    for bk in range(2):
                        eo = e0 + half * 1024 + bk * 512
                        mm.append(nc.tensor.matmul(
                            ps_h[half * 64:(half + 1) * 64,
                                 bk * 512:(bk + 1) * 512],
                            lhsT=wm1_sb, rhs=ef_sb[:, eo:eo + 512],
                            start=True, stop=True))
                for x, y in zip(mm, mm[1:]):
                    _dep(y, x)
                h1 = mlpp.tile([128, 1024], bf16, tag="h1")
                nc.scalar.activation(h1[:], ps_h[:],
                                     mybir.ActivationFunctionType.Silu)
                ps_h2 = psm.tile([128, 1024], f32, tag="ps_h", space="PSUM")
                mm = []
                for half in range(2):
                    hw = wm2_sb[half * 64:(half + 1) * 64, :]
                    for bk in range(2):
                        mm.append(nc.tensor.matmul(
                            ps_h2[half * 64:(half + 1) * 64,
                                  bk * 512:(bk + 1) * 512],
                            lhsT=hw,
                            rhs=h1[half * 64:(half + 1) * 64,
                                   bk * 512:(bk + 1) * 512],
                            start=True, stop=True))
                for x, y in zip(mm, mm[1:]):
                    _dep(y, x)
                h2 = mlpp.tile([128, 1024], bf16, tag="h2")
                nc.scalar.activation(h2[:], ps_h2[:],
                                     mybir.ActivationFunctionType.Silu)
                ps_mix = psm.tile([128, 64], f32, tag="ps_h", space="PSUM")
                mm = []
                for j in range(16):
                    half, jj = j // 8, j % 8
                    mm.append(nc.tensor.matmul(
                        ps_mix[:, j * 4:j * 4 + 4],
                        lhsT=h2[half * 64:(half + 1) * 64,
                                jj * 128:(jj + 1) * 128],
                        rhs=wm3_sb[half * 64:(half + 1) * 64, :],
                        start=True, stop=True))
                for x, y in zip(mm, mm[1:]):
                    _dep(y, x)
                mix_v = ps_mix[:].rearrange("p (c m) -> p c m", m=4)

                # -- per-edge block scalars: [q0, a*q3 (3), q2, a*q1 (3)]
                q8t = qp.tile([128, 16, 8], bf16, tag="q8t")
                nc.vector.tensor_copy(q8t[:, :, 0], mix_v[:, :, 0])
                nc.vector.tensor_tensor(
                    out=q8t[:, :, 1:4], in0=aT_v[:, c0:c0 + 16, :],
                    in1=mix_v[:, :, 3:4].to_broadcast([128, 16, 3]),
                    op=mybir.AluOpType.mult)
                nc.vector.tensor_copy(q8t[:, :, 4], mix_v[:, :, 2])
                nc.vector.tensor_tensor(
                    out=q8t[:, :, 5:8], in0=aT_v[:, c0:c0 + 16, :],
                    in1=mix_v[:, :, 1:2].to_broadcast([128, 16, 3]),
                    op=mybir.AluOpType.mult)

                # -- one-hot for 16 chunks, then dual-broadcast scatter rhs
                oh = ohp.tile([128, 16, 16], bf16, tag="oh")
                nc.vector.tensor_tensor(
                    out=oh[:], in0=iota_v,
                    in1=rloc_sb[:, c0:c0 + 16].to_broadcast([128, 16, 16]),
                    op=mybir.AluOpType.is_equal)
                hall = ohp.tile([128, 16, 8, 16], bf16, tag="hall")
                nc.vector.tensor_tensor(
                    out=hall[:],
                    in0=oh[:].unsqueeze(2).to_broadcast([128, 16, 8, 16]),
                    in1=q8t[:].unsqueeze(3).to_broadcast([128, 16, 8, 16]),
                    op=mybir.AluOpType.mult)

                # -- scatter-accumulate 16 chunks into agg PSUM
                agg = psagg.tile([128, 1024], f32, tag="agg", space="PSUM")
                bank_mms = [[], []]
                for j in range(16):
                    w = j // 2
                    bank = w // 4
                    fg_h, jl = fg_of_j[j]
                    hj = hall[:, j, :, :].rearrange("p b s -> p (b s)")
                    wb = w * 128
                    # blocks: [G0 G1 G2 G3 | M | D1 D2 D3]
                    # agg cols per window: [s0 sA sB sC m_x dot m_y m_z]
                    for (comp, col, ncol, lo_) in (
                        (0, wb + 0, 64, 0),
                        (1, wb + 64, 32, 64),
                        (2, wb + 80, 16, 96),
                        (2, wb + 96, 16, 64),
                        (3, wb + 80, 16, 112),
                        (3, wb + 112, 16, 64),
                    ):
                        mmi = nc.tensor.matmul(
                            agg[:, col:col + ncol],
                            lhsT=fg_h[:, jl, comp * 128:(comp + 1) * 128],
                            rhs=hj[:, lo_:lo_ + ncol],
                            start=(len(bank_mms[bank]) == 0), stop=False)
                        bank_mms[bank].append(mmi)
                for bank in range(2):
                    mms = bank_mms[bank]
                    mms[-1].ins.stop_tensor_calc = True
                    for m in mms[1:]:
                        _dep(m, mms[0])
                    for m in mms[:-1]:
                        _dep(mms[-1], m)
                # evict with (window, block, slot) -> (block, window, slot)
                agg_sb = evp.tile([128, 1024], bf16, tag="agg_sb")
                nc.scalar.copy(
                    agg_sb[:].rearrange("p (b a c) -> p b a c", b=8, a=8, c=16),
                    agg[:].rearrange("p (a b c) -> p b a c", a=8, b=8, c=16))
                # -- postmp with folded weights
                o_ps = pso.tile([128, 512], f32, tag="ops", space="PSUM")
                och = []
                for (ocol, blk, wblk, st) in (
                    (0, 0, 0, True), (0, 5, 1, False),      # o_s = s0@A + dot@B
                    (128, 4, 2, True), (128, 1, 3, False),  # o_vx = m_x@C + sA@D
                    (256, 6, 2, True), (256, 2, 3, False),  # o_vy = m_y@C + sB@D
                    (384, 7, 2, True), (384, 3, 3, False),  # o_vz = m_z@C + sC@D
                ):
                    och.append(nc.tensor.matmul(
                        o_ps[:, ocol:ocol + 128],
                        lhsT=agg_sb[:, blk * 128:(blk + 1) * 128],
                        rhs=wcomb_sb[:, wblk * 128:(wblk + 1) * 128],
                        start=st, stop=not st))
                for x, y in zip(och, och[1:]):
                    _dep(y, x)
                out_sb = evp.tile([128, 512], bf16, tag="out_sb")
                nc.vector.tensor_copy(
                    out_sb[:].rearrange("p (d c) -> p c d", c=4),
                    o_ps[:].rearrange("p (c d) -> p c d", c=4))
                eng = nc.sync if t == NT - 1 else nc.scalar
                eng.dma_start(nout[t * 128:(t + 1) * 128, :], out_sb[:])

                # -- interleaved sc tiles (species linear, flipped so node
                # runs sit on the free dim where any split offset is legal)
                for gt in sc_of_t[t]:
                    ps_sc = pso.tile([128, 512], f32, tag="ops", space="PSUM")
                    chain = []
                    for (r0, nr, sp) in runs_of_tile[gt]:
                        for comp in range(4):
                            wsc = (wsc0_sb if comp == 0 else wsc1_sb)[
                                :, sp * 128:(sp + 1) * 128]
                            rhs = nfc[comp][:, gt * 128 + r0:
                                            gt * 128 + r0 + nr]
                            chain.append(nc.tensor.matmul(
                                ps_sc[:, comp * 128 + r0:
                                      comp * 128 + r0 + nr],
                                lhsT=wsc, rhs=rhs, start=True, stop=True))
                    for x, y in zip(chain, chain[1:]):
                        _dep(y, x)
                    sc_sb = sa2.tile([128, 128, 4], bf16, tag="sc_sb")
                    src = ps_sc[:].rearrange("p (m n) -> p n m", m=4)
                    if gt % 2 == 0:
                        nc.vector.tensor_copy(sc_sb[:], src)
                    else:
                        nc.scalar.copy(sc_sb[:], src)
                    nc.gpsimd.dma_start(
                        sc_out[gt * 128:(gt + 1) * 128, :],
                        sc_sb[:].rearrange("p n m -> p (n m)"))

    nc.compile()
    return nc


_IN_CHILD = False


def _child_entry(q, inputs):
    """Fresh-process fallback: the PJRT/axon runtime occasionally fails a
    run and stays wedged for the process; a clean process recovers."""
    global _IN_CHILD
    _IN_CHILD = True
    try:
        q.put(("ok", kernel(**inputs)))
    except BaseException as e:  # noqa: BLE001
        q.put(("err", repr(e)))


def kernel(**inputs):
    meta, per_core, shared, unshard = _host_prep(**inputs)
    key = (meta["NT"], meta["NCH"], meta["NSLOT"], meta["runs_of_tile"])
    if key not in _CACHE:
        _CACHE[key] = _build(meta)
    nc = _CACHE[key]
    in_maps = [dict(pc, **shared) for pc in per_core]
    res = None
    err = None
    for attempt in range(2):
        try:
            res = run_bass_kernel_spmd(nc, in_maps,
                                       core_ids=list(range(NCORES)))
            break
        except Exception as e:
            err = e
            try:
                import jax
                jax.clear_caches()
                import jax.extend.backend as _jeb
                _jeb.clear_backends()
            except Exception:
                pass
    if res is None:
        if _IN_CHILD:
            raise err
        import multiprocessing as mp
        ctx = mp.get_context("spawn")
        for attempt in range(3):
            q = ctx.Queue()
            p = ctx.Process(target=_child_entry, args=(q, inputs))
            p.start()
            try:
                status, payload = q.get(timeout=1800)
            except Exception:
                status, payload = "err", "child timeout"
            p.join(timeout=60)
            if p.is_alive():
                p.terminate()
            if status == "ok":
                return payload
        raise err

    NT, NSLOT = unshard["NT"], unshard["NSLOT"]
    core_of, win_of, slot_of = (unshard["core_of"], unshard["win_of"],
                                unshard["slot_of"])
    node_of_scslot = unshard["node_of_scslot"]
    node_out = np.zeros((N, 128, 4), np.float32)
    sc = np.zeros((N, 128, 4), np.float32)
    rows_all = win_of * W + slot_of          # per-core row in nout
    for k in range(NCORES):
        nk = np.nonzero(core_of == k)[0]
        no_k = np.asarray(res.results[k]["nout"], dtype=np.float32)
        node_out[nk] = no_k[rows_all[nk]].reshape(-1, 128, 4)
        valid = node_of_scslot[k] >= 0
        scT = np.asarray(res.results[k]["sc_out"], dtype=np.float32)
        scT = (scT.reshape(NSLOT // 128, 128, 128, 4)
               .transpose(1, 0, 2, 3).reshape(128, NSLOT, 4))
        sc[node_of_scslot[k][valid]] = scT[:, valid, :].transpose(1, 0, 2)
    return node_out, sc


# revision 66
# speedup vs baseline: 1.0029x; 1.0029x over previous
"""Trainium2 Bass kernel for nn_AgnosticResidualInteractionBlock (GNN message passing).

Strategy (8 NeuronCores, receiver-node graph partition; all constant scales and
the pre-message linear W_pre are folded into host-side weights):
  - Host: pack nodes into 16-receiver *windows*, each with EXACTLY 16 nodes and
    256 incoming edges (perfect bin-packing with swap repair; falls back to the
    greedy <=256 packer), so the per-core edge stream has zero padding;
    pre-gather raw sender features into a per-core bf16 edge stream; fold
    W_pre @ W_post into four combined 128x128 matrices so the device scatters
    RAW features and applies one output linear per block; pack all small
    constants into one DMA.
  - Device (SPMD, one Bass program on 8 cores), per 128-receiver tile:
      radial MLP -> per-edge mix scalars; one batched is_equal builds the
      16-chunk one-hot slab, one dual-broadcast multiply expands it into the
      8-block scatter rhs; six matmuls per chunk scatter-accumulate into PSUM
      (fg streamed in two half-tile DMAs aligned with the two PSUM banks);
      post-message linear with folded weights. The species linear (sc) is
      interleaved into iterations 2..NT-3 so its compute and stores hide under
      the edge stream. DMA rings are split: loads on SP (sync), nout stores on
      Act (scalar), sc stores on SWDGE (gpsimd) so stores never stall loads.
  - Host: map receiver slots back to node ids, cast outputs to f32.
"""
import sys
import numpy as np

try:
    import concourse.bacc as bacc
except ImportError:  # pragma: no cover
    sys.path.insert(0, "/opt/trn_rl_repo")
    import concourse.bacc as bacc

import ml_dtypes

import concourse.bass as bass
import concourse.mybir as mybir
import concourse.tile as tile
from concourse.bass_utils import run_bass_kernel_spmd

BF16 = ml_dtypes.bfloat16
N, C, E, S = 16384, 128, 262144, 10
NCORES = 8
AVG = 16.0
INV_C = 1.0 / np.sqrt(C)
INV2C = 1.0 / np.sqrt(2 * C)
INV_SQRT3 = 1.0 / np.sqrt(3.0)
W = 16                  # receivers per window
WCAP = 256              # edge slots per window (2 chunks)

_CACHE = {}


def _cpack_layout(NCH):
    """Column offsets of the packed small-constant tensor [128, CP]."""
    o = {}
    o["wm1"] = 0            # [8, 64] on partitions 0:8
    o["wm2"] = 64           # [128, 64]
    o["wm3"] = 128          # [128, 4]
    o["wcomb"] = 132        # [128, 512]
    o["iota"] = 644         # [128, 256]
    o["rloc"] = 900         # [128, NCH]
    o["aT"] = 900 + NCH     # [128, NCH*3]
    o["end"] = 900 + 4 * NCH
    return o


def _snake_repair(items_deg, groups, per, target, rng, iters=300000):
    """Partition `len(items_deg)` items into `groups` groups of exactly `per`
    items with degree-sum exactly `target` each. Returns idx [groups, per] of
    item indices, or None."""
    order = np.argsort(-items_deg, kind="stable")
    idx = np.empty((groups, per), np.int64)
    for r in range(per):
        chunk = order[r * groups:(r + 1) * groups]
        idx[:, r] = chunk if r % 2 == 0 else chunk[::-1]
    sums = items_deg[idx].sum(1)
    for _ in range(iters):
        dev = sums - target
        over = np.nonzero(dev > 0)[0]
        if len(over) == 0:
            return idx
        under = np.nonzero(dev < 0)[0]
        a = over[np.argmax(dev[over])]
        b = under[np.argmin(dev[under])]
        for attempt in range(2):
            da, db = items_deg[idx[a]], items_deg[idx[b]]
            diff = da[:, None] - db[None, :]
            cost = np.abs(dev[a] - diff) + np.abs(dev[b] + diff)
            i, j = np.unravel_index(np.argmin(cost), cost.shape)
            if cost[i, j] < abs(dev[a]) + abs(dev[b]):
                idx[a, i], idx[b, j] = idx[b, j], idx[a, i]
                sums[a] = items_deg[idx[a]].sum()
                sums[b] = items_deg[idx[b]].sum()
                break
            a = rng.choice(over)
            b = rng.choice(under)
        else:
            continue
    return None


def _balanced_core_assign(deg, node_specie):
    """Assign nodes to cores: per-species counts equal (up to +-1 padded to a
    common T[s]) AND per-core degree sums exactly E/NCORES. Returns
    (core_of, T) or None."""
    target = E // NCORES
    rng = np.random.default_rng(1)
    core_nodes = [[] for _ in range(NCORES)]
    counts = np.zeros(NCORES, np.int64)
    T = np.zeros(S, np.int64)
    for s in range(S):
        ns = np.nonzero(node_specie == s)[0]
        ns = ns[np.argsort(-deg[ns], kind="stable")]
        T[s] = -(-len(ns) // NCORES)
        q, r = divmod(len(ns), NCORES)
        # ceil-share goes to the cores with the lowest running totals so
        # every core ends with exactly N/NCORES nodes overall
        share = np.full(NCORES, q, np.int64)
        share[np.argsort(counts, kind="stable")[:r]] += 1
        pos = 0
        take = share.copy()
        i = 0
        while pos < len(ns):
            chunk = ns[pos:pos + NCORES]
            if i % 2 == 1:
                chunk = chunk[::-1]
            for n in chunk:
                k = int(np.argmax(take))
                core_nodes[k].append(n)
                take[k] -= 1
            pos += len(chunk)
            i += 1
        counts += share
    core_nodes = [np.array(c) for c in core_nodes]
    cdeg = np.array([deg[c].sum() for c in core_nodes])
    # repair with same-species swaps between cores
    for _ in range(200000):
        dev = cdeg - target
        over = np.nonzero(dev > 0)[0]
        if len(over) == 0:
            break
        under = np.nonzero(dev < 0)[0]
        a = over[np.argmax(dev[over])]
        b = under[np.argmin(dev[under])]
        best = None
        for s in range(S):
            ia = np.nonzero(node_specie[core_nodes[a]] == s)[0]
            ib = np.nonzero(node_specie[core_nodes[b]] == s)[0]
            if len(ia) == 0 or len(ib) == 0:
                continue
            da = deg[core_nodes[a][ia]]
            db = deg[core_nodes[b][ib]]
            diff = da[:, None] - db[None, :]
            cost = np.abs(dev[a] - diff) + np.abs(dev[b] + diff)
            i, j = np.unravel_index(np.argmin(cost), cost.shape)
            if best is None or cost[i, j] < best[0]:
                best = (cost[i, j], ia[i], ib[j])
        if best is None or best[0] >= abs(dev[a]) + abs(dev[b]):
            a = rng.choice(over)
            b = rng.choice(under)
            continue
        _, i, j = best
        na, nb = core_nodes[a][i], core_nodes[b][j]
        core_nodes[a][i], core_nodes[b][j] = nb, na
        cdeg[a] = deg[core_nodes[a]].sum()
        cdeg[b] = deg[core_nodes[b]].sum()
    else:
        pass
    if np.any(cdeg != target):
        return None
    core_of = np.zeros(N, np.int64)
    for k in range(NCORES):
        core_of[core_nodes[k]] = k
    return core_of, T


def _perfect_pack(deg, node_specie):
    """Pack nodes into NCORES*128 windows with EXACTLY 16 nodes and 256 edges
    each (zero padding), with per-core species counts balanced to a common
    vector T. Returns (core_of, win_of, slot_of, T) or None."""
    nwin = NCORES * 128
    if deg.sum() != nwin * WCAP or len(deg) != nwin * W or deg.max() > WCAP:
        return None
    res = _balanced_core_assign(deg, node_specie)
    if res is None:
        return None
    core_of, T = res
    win_of = np.zeros(N, np.int64)
    slot_of = np.zeros(N, np.int64)
    rng = np.random.default_rng(2)
    for k in range(NCORES):
        nk = np.nonzero(core_of == k)[0]
        idx = _snake_repair(deg[nk], 128, W, WCAP, rng)
        if idx is None:
            return None
        for w in range(128):
            for s in range(W):
                n = nk[idx[w, s]]
                win_of[n] = w
                slot_of[n] = s
    return core_of, win_of, slot_of, T


def _pack_windows(deg, nwc):
    """Greedy best-fit-decreasing fallback: nodes -> (core, window) with <=16
    nodes and <=256 edges per window. Returns tuple or None."""
    nwin = NCORES * nwc
    cap = np.full(nwin, WCAP, np.int64)
    slots = np.full(nwin, W, np.int64)
    core_of = np.zeros(N, np.int64)
    win_of = np.zeros(N, np.int64)
    slot_of = np.zeros(N, np.int64)
    order = np.argsort(-deg, kind="stable")
    eligible = np.ones(nwin, bool)
    for n in order:
        d = deg[n]
        c = np.where(eligible, cap, -1)
        w = int(np.argmax(c))
        if c[w] < d:
            return None
        cap[w] -= d
        slots[w] -= 1
        slot_of[n] = W - 1 - slots[w]
        if slots[w] == 0:
            eligible[w] = False
        core_of[n] = w // nwc
        win_of[n] = w % nwc
    return core_of, win_of, slot_of


def _host_prep(node_specie, node_feats, edge_attrs, edge_feats, senders, receivers,
               W_sc0, W_sc1, W_pre0, W_pre1, W_mlp1, W_mlp2, W_mlp3, W_post0, W_post1):
    senders = np.asarray(senders).astype(np.int64)
    receivers = np.asarray(receivers).astype(np.int64)
    node_specie = np.asarray(node_specie).astype(np.int64)
    node_feats = np.asarray(node_feats, dtype=np.float32)
    edge_attrs = np.asarray(edge_attrs, dtype=np.float32)
    edge_feats = np.asarray(edge_feats, dtype=np.float32)

    # ---- window packing (receiver partition balanced by degree) ------------
    deg = np.bincount(receivers, minlength=N)
    T = None
    res = _perfect_pack(deg, node_specie)
    if res is not None:
        core_of, win_of, slot_of, T = res
        nwc = 128
    else:
        nwc = 131
        while True:
            res = _pack_windows(deg, nwc)
            if res is not None:
                break
            nwc += 2
        core_of, win_of, slot_of = res
    NT = -(-nwc // 8)            # tiles per core
    NWC = NT * 8                 # padded windows per core
    NCH = NT * 16                # chunks per core
    EPADP = NCH * 128            # edge slots per core

    # ---- edge placement into per-core slot streams -------------------------
    er_core = core_of[receivers]
    er_win = win_of[receivers]
    gw = er_core * NWC + er_win
    eorder = np.argsort(gw, kind="stable")
    gw_s = gw[eorder]
    within = np.arange(E) - np.searchsorted(gw_s, gw_s)  # index within window
    slot_global = gw_s * WCAP + within                   # global slot id
    # per-core slot arrays
    send_slot = np.full(NCORES * EPADP, -1, np.int64)
    rloc_slot = np.full(NCORES * EPADP, -1.0, np.float32)
    a_slot = np.zeros((NCORES * EPADP, 3), np.float32)
    ef_slot = np.zeros((NCORES * EPADP, 8), np.float32)
    send_slot[slot_global] = senders[eorder]
    rloc_slot[slot_global] = slot_of[receivers[eorder]].astype(np.float32)
    a_slot[slot_global] = edge_attrs[eorder][:, 1:4]
    ef_slot[slot_global] = edge_feats[eorder]

    # ---- species permutation per core (sc stage) ---------------------------
    # Slot layout is a sequence of species runs at core-invariant offsets;
    # tiles may span species (the program emits one matmul set per run).
    nodes_of_core = [np.nonzero(core_of == k)[0] for k in range(NCORES)]
    if T is None:
        spec_counts = np.zeros((NCORES, S), np.int64)
        for k in range(NCORES):
            spec_counts[k] = np.bincount(node_specie[nodes_of_core[k]],
                                         minlength=S)
        T = np.maximum(1, -(-spec_counts.max(axis=0) // 128)) * 128
    run_off = np.concatenate([[0], np.cumsum(T)])
    NGT = -(-int(run_off[-1]) // 128)
    NSLOT = NGT * 128
    runs_of_tile = []
    for gt in range(NGT):
        lo_t, hi_t = gt * 128, (gt + 1) * 128
        rs = []
        for s in range(S):
            a, b = max(lo_t, run_off[s]), min(hi_t, run_off[s + 1])
            if b > a:
                rs.append((int(a - lo_t), int(b - a), int(s)))
        runs_of_tile.append(tuple(rs))
    runs_of_tile = tuple(runs_of_tile)
    node_of_scslot = -np.ones((NCORES, NSLOT), np.int64)   # global node ids
    for k in range(NCORES):
        nk = nodes_of_core[k]
        sp = node_specie[nk]
        for s in range(S):
            g = nk[sp == s]
            node_of_scslot[k, run_off[s] + np.arange(len(g))] = g

    # ---- weights (scales folded) -------------------------------------------
    sc_post = INV2C / AVG
    Wp0 = np.asarray(W_pre0, np.float64) * INV_C
    Wp1 = np.asarray(W_pre1, np.float64) * INV_C
    Wo0 = np.asarray(W_post0, np.float64) * sc_post
    Wo1 = np.asarray(W_post1, np.float64) * sc_post
    A = Wp0 @ Wo0[:, 0, :]
    B = (Wp1 @ Wo0[:, 1, :]) * INV_SQRT3
    Cm = Wp1 @ Wo1[:, 0, :]
    D = Wp0 @ Wo1[:, 1, :]
    wcomb = np.concatenate([A, B, Cm, D], axis=1).astype(np.float32)
    wsc0T = ((np.asarray(W_sc0, np.float32) * INV_C)
             .transpose(1, 0, 2).reshape(128, S * 128)).astype(BF16)
    wsc1T = ((np.asarray(W_sc1, np.float32) * INV_C)
             .transpose(1, 0, 2).reshape(128, S * 128)).astype(BF16)

    # ---- packed small constants -------------------------------------------
    co = _cpack_layout(NCH)
    cpack = np.zeros((128, co["end"]), np.float32)
    cpack[0:8, co["wm1"]:co["wm1"] + 64] = np.asarray(W_mlp1, np.float32) / np.sqrt(8.0)
    cpack[0:64, co["wm2"]:co["wm2"] + 64] = np.asarray(W_mlp2, np.float32) * 0.125
    cpack[64:128, co["wm2"]:co["wm2"] + 64] = cpack[0:64, co["wm2"]:co["wm2"] + 64]
    cpack[0:64, co["wm3"]:co["wm3"] + 4] = np.asarray(W_mlp3, np.float32) * 0.125
    cpack[64:128, co["wm3"]:co["wm3"] + 4] = cpack[0:64, co["wm3"]:co["wm3"] + 4]
    cpack[:, co["wcomb"]:co["wcomb"] + 512] = wcomb
    cpack[:, co["iota"]:co["iota"] + 256] = np.tile(
        np.arange(W, dtype=np.float32), (128, 16))
    cpack_c = cpack.astype(BF16)   # per-core copies get rloc/aT appended below

    # ---- per-core device arrays -------------------------------------------
    nfb2 = np.ascontiguousarray(
        node_feats.transpose(0, 2, 1).reshape(N, 512)).astype(BF16)
    nfb3 = np.concatenate([nfb2, np.zeros((1, 512), BF16)], axis=0)
    per_core = []
    for k in range(NCORES):
        sl = slice(k * EPADP, (k + 1) * EPADP)
        snd = send_slot[sl]
        rows = nfb3[snd]                       # [-1] -> zero row
        rows[snd < 0] = 0
        fgS = np.ascontiguousarray(
            rows.reshape(NCH, 128, 512).transpose(1, 0, 2).reshape(128, NCH * 512))
        cpk = cpack_c.copy()
        cpk[:, co["rloc"]:co["rloc"] + NCH] = (
            rloc_slot[sl].reshape(NCH, 128).T.astype(BF16))
        cpk[:, co["aT"]:co["aT"] + NCH * 3] = (
            a_slot[sl].reshape(NCH, 128, 3).transpose(1, 0, 2)
            .reshape(128, NCH * 3).astype(BF16))
        efT = np.ascontiguousarray(ef_slot[sl].T).astype(BF16)
        nfT = np.zeros((4, 128, NSLOT), np.float32)
        valid = node_of_scslot[k] >= 0
        nfT[:, :, valid] = node_feats[node_of_scslot[k][valid]].transpose(2, 1, 0)
        per_core.append(dict(fgS=fgS, cpk=cpk, efT=efT, nfT=nfT.astype(BF16)))

    shared = dict(wsc0T=wsc0T, wsc1T=wsc1T)
    meta = dict(NT=NT, NCH=NCH, NSLOT=NSLOT, runs_of_tile=runs_of_tile)
    unshard = dict(core_of=core_of, win_of=win_of, slot_of=slot_of,
                   node_of_scslot=node_of_scslot, NT=NT, NSLOT=NSLOT)
    return meta, per_core, shared, unshard


def _dep(later, earlier):
    tile.add_dep_helper(later.ins, earlier.ins, sync=False, reason="order")


def _build(meta):
    NT, NCH, NSLOT = meta["NT"], meta["NCH"], meta["NSLOT"]
    runs_of_tile = meta["runs_of_tile"]
    NGT = NSLOT // 128
    EPADP = NCH * 128
    f32, bf16 = mybir.dt.float32, mybir.dt.bfloat16
    co = _cpack_layout(NCH)

    nc = bacc.Bacc("TRN2", target_bir_lowering=False)
    fgS = nc.dram_tensor("fgS", [128, NCH * 512], bf16, kind="ExternalInput")
    cpk = nc.dram_tensor("cpk", [128, co["end"]], bf16, kind="ExternalInput")
    efT = nc.dram_tensor("efT", [8, EPADP], bf16, kind="ExternalInput")
    nfT = nc.dram_tensor("nfT", [4, 128, NSLOT], bf16, kind="ExternalInput")
    wsc0T = nc.dram_tensor("wsc0T", [128, S * 128], bf16, kind="ExternalInput")
    wsc1T = nc.dram_tensor("wsc1T", [128, S * 128], bf16, kind="ExternalInput")
    # transposed: row gt*128+out_ch, col node*4+comp; host untransposes
    sc_out = nc.dram_tensor("sc_out", [NGT * 128, 512], bf16,
                            kind="ExternalOutput")
    nout = nc.dram_tensor("nout", [NT * 128, 512], bf16, kind="ExternalOutput")

    # distribute the NGT sc tiles over main iterations 2..NT-3, singles
    # first and doubles late (compute slack grows once the prologue clears)
    sc_of_t = [[] for _ in range(NT)]
    lo = min(2, NT - 1)
    hi = max(lo + 1, NT - 2)            # exclusive
    slots = list(range(lo, hi))
    g = 0
    for t in slots:
        if g < NGT:
            sc_of_t[t].append(g)
            g += 1
    for t in reversed(slots):
        if g < NGT:
            sc_of_t[t].append(g)
            g += 1
    while g < NGT:                       # fallback if very few slots
        sc_of_t[hi - 1].append(g)
        g += 1

    with tile.TileContext(nc) as tc:
        with tc.tile_pool(name="cst", bufs=1) as cst, \
             tc.tile_pool(name="sa2", bufs=3) as sa2, \
             tc.tile_pool(name="fg", bufs=8) as fgp, \
             tc.tile_pool(name="mlp", bufs=2) as mlpp, \
             tc.tile_pool(name="qp", bufs=3) as qp, \
             tc.tile_pool(name="ohp", bufs=2) as ohp, \
             tc.tile_pool(name="ev", bufs=2) as evp, \
             tc.tile_pool(name="psm", bufs=2, space="PSUM") as psm, \
             tc.tile_pool(name="psagg", bufs=1, space="PSUM") as psagg, \
             tc.tile_pool(name="pso", bufs=2, space="PSUM") as pso:
            # ---- prologue loads (SP ring; all zero-dep) --------------------
            cpk_sb = cst.tile([128, co["end"]], bf16)
            nc.sync.dma_start(cpk_sb[:], cpk[:])
            ef_sb = cst.tile([8, EPADP], bf16)
            nc.sync.dma_start(ef_sb[:], efT[:])
            wm1_sb = cpk_sb[0:8, co["wm1"]:co["wm1"] + 64]
            wm2_sb = cpk_sb[:, co["wm2"]:co["wm2"] + 64]
            wm3_sb = cpk_sb[:, co["wm3"]:co["wm3"] + 4]
            wcomb_sb = cpk_sb[:, co["wcomb"]:co["wcomb"] + 512]
            iota_v = cpk_sb[:, co["iota"]:co["iota"] + 256].rearrange(
                "p (c s) -> p c s", s=16)
            rloc_sb = cpk_sb[:, co["rloc"]:co["rloc"] + NCH]
            aT_v = cpk_sb[:, co["aT"]:co["aT"] + NCH * 3].rearrange(
                "p (c k) -> p c k", k=3)

            wsc_loaded = False
            nfc = []

            # ---- main loop: one 128-receiver tile per iteration ------------
            for t in range(NT):
                e0 = t * 2048          # first edge slot of tile
                c0 = t * 16            # first chunk of tile
                # last tile streams in quarters so its scatter overlaps the
                # final transfers; earlier tiles in halves
                csz = 4 if t == NT - 1 else 8
                fg_of_j = []
                for h in range(16 // csz):
                    fh = fgp.tile([128, csz, 512], bf16, tag="fg")
                    nc.sync.dma_start(
                        fh[:].rearrange("p a b -> p (a b)"),
                        fgS[:, (c0 + csz * h) * 512:
                            (c0 + csz * (h + 1)) * 512])
                    fg_of_j += [(fh, jl) for jl in range(csz)]
                if t == 1:
                    # sc-stage loads ride behind the first two fg tiles
                    wsc0_sb = cst.tile([128, S * 128], bf16)
                    nc.sync.dma_start(wsc0_sb[:], wsc0T[:])
                    wsc1_sb = cst.tile([128, S * 128], bf16)
                    nc.sync.dma_start(wsc1_sb[:], wsc1T[:])
                    for comp in range(4):
                        t_ = cst.tile([128, NSLOT], bf16, name=f"nfc{comp}")
                        nc.sync.dma_start(t_[:], nfT[comp, :, :])
                        nfc.append(t_)

                # -- radial MLP for this tile's 2048 edges (2x1024 packed)
                ps_h = psm.tile([128, 1024], f32, tag="ps_h", space="PSUM")
                mm = []
                for half in range(2):
                    for bk in range(2):
                        eo = e0 + half * 1024 + bk * 512
                        mm.append(nc.tensor.matmul(
                            ps_h[half * 64:(half + 1) * 64,
                                 bk * 512:(bk + 1) * 512],
                            lhsT=wm1_sb, rhs=ef_sb[:, eo:eo + 512],
                            start=True, stop=True))
                for x, y in zip(mm, mm[1:]):
                    _dep(y, x)
                h1 = mlpp.tile([128, 1024], bf16, tag="h1")
                nc.scalar.activation(h1[:], ps_h[:],
                                     mybir.ActivationFunctionType.Silu)
                ps_h2 = psm.tile([128, 1024], f32, tag="ps_h", space="PSUM")
                mm = []
                for half in range(2):
                    hw = wm2_sb[half * 64:(half + 1) * 64, :]
                    for bk in range(2):
                        mm.append(nc.tensor.matmul(
                            ps_h2[half * 64:(half + 1) * 64,
                                  bk * 512:(bk + 1) * 512],
                            lhsT=hw,
                            rhs=h1[half * 64:(half + 1) * 64,
                                   bk * 512:(bk + 1) * 512],
                            start=True, stop=True))
                for x, y in zip(mm, mm[1:]):
                    _dep(y, x)
                h2 = mlpp.tile([128, 1024], bf16, tag="h2")
                nc.scalar.activation(h2[:], ps_h2[:],
                                     mybir.ActivationFunctionType.Silu)
                ps_mix = psm.tile([128, 64], f32, tag="ps_h", space="PSUM")
                mm = []
                for j in range(16):
                    half, jj = j // 8, j % 8
                    mm.append(nc.tensor.matmul(
                        ps_mix[:, j * 4:j * 4 + 4],
                        lhsT=h2[half * 64:(half + 1) * 64,
                                jj * 128:(jj + 1) * 128],
                        rhs=wm3_sb[half * 64:(half + 1) * 64, :],
                        start=True, stop=True))
                for x, y in zip(mm, mm[1:]):
                    _dep(y, x)
                mix_v = ps_mix[:].rearrange("p (c m) -> p c m", m=4)

                # -- per-edge block scalars: [q0, a*q3 (3), q2, a*q1 (3)]
                q8t = qp.tile([128, 16, 8], bf16, tag="q8t")
                nc.vector.tensor_copy(q8t[:, :, 0], mix_v[:, :, 0])
                nc.vector.tensor_tensor(
                    out=q8t[:, :, 1:4], in0=aT_v[:, c0:c0 + 16, :],
                    in1=mix_v[:, :, 3:4].to_broadcast([128, 16, 3]),
                    op=mybir.AluOpType.mult)
                nc.vector.tensor_copy(q8t[:, :, 4], mix_v[:, :, 2])
                nc.vector.tensor_tensor(
                    out=q8t[:, :, 5:8], in0=aT_v[:, c0:c0 + 16, :],
                    in1=mix_v[:, :, 1:2].to_broadcast([128, 16, 3]),
                    op=mybir.AluOpType.mult)

                # -- one-hot for 16 chunks, then dual-broadcast scatter rhs
                oh = ohp.tile([128, 16, 16], bf16, tag="oh")
                nc.vector.tensor_tensor(
                    out=oh[:], in0=iota_v,
                    in1=rloc_sb[:, c0:c0 + 16].to_broadcast([128, 16, 16]),
                    op=mybir.AluOpType.is_equal)
                hall = ohp.tile([128, 16, 8, 16], bf16, tag="hall")
                nc.vector.tensor_tensor(
                    out=hall[:],
                    in0=oh[:].unsqueeze(2).to_broadcast([128, 16, 8, 16]),
                    in1=q8t[:].unsqueeze(3).to_broadcast([128, 16, 8, 16]),
                    op=mybir.AluOpType.mult)

                # -- scatter-accumulate 16 chunks into agg PSUM
                agg = psagg.tile([128, 1024], f32, tag="agg", space="PSUM")
                bank_mms = [[], []]
                for j in range(16):
                    w = j // 2
                    bank = w // 4
                    fg_h, jl = fg_of_j[j]
                    hj = hall[:, j, :, :].rearrange("p b s -> p (b s)")
                    wb = w * 128
                    # blocks: [G0 G1 G2 G3 | M | D1 D2 D3]
                    # agg cols per window: [s0 sA sB sC m_x dot m_y m_z]
                    for (comp, col, ncol, lo_) in (
                        (0, wb + 0, 64, 0),
                        (1, wb + 64, 32, 64),
                        (2, wb + 80, 16, 96),
                        (2, wb + 96, 16, 64),
                        (3, wb + 80, 16, 112),
                        (3, wb + 112, 16, 64),
                    ):
                        mmi = nc.tensor.matmul(
                            agg[:, col:col + ncol],
                            lhsT=fg_h[:, jl, comp * 128:(comp + 1) * 128],
                            rhs=hj[:, lo_:lo_ + ncol],
                            start=(len(bank_mms[bank]) == 0), stop=False)
                        bank_mms[bank].append(mmi)
                for bank in range(2):
                    mms = bank_mms[bank]
                    mms[-1].ins.stop_tensor_calc = True
                    for m in mms[1:]:
                        _dep(m, mms[0])
                    for m in mms[:-1]:
                        _dep(mms[-1], m)
                # evict with (window, block, slot) -> (block, window, slot)
                agg_sb = evp.tile([128, 1024], bf16, tag="agg_sb")
                nc.scalar.copy(
                    agg_sb[:].rearrange("p (b a c) -> p b a c", b=8, a=8, c=16),
                    agg[:].rearrange("p (a b c) -> p b a c", a=8, b=8, c=16))
                # -- postmp with folded weights
                o_ps = pso.tile([128, 512], f32, tag="ops", space="PSUM")
                och = []
                for (ocol, blk, wblk, st) in (
                    (0, 0, 0, True), (0, 5, 1, False),      # o_s = s0@A + dot@B
                    (128, 4, 2, True), (128, 1, 3, False),  # o_vx = m_x@C + sA@D
                    (256, 6, 2, True), (256, 2, 3, False),  # o_vy = m_y@C + sB@D
                    (384, 7, 2, True), (384, 3, 3, False),  # o_vz = m_z@C + sC@D
                ):
                    och.append(nc.tensor.matmul(
                        o_ps[:, ocol:ocol + 128],
                        lhsT=agg_sb[:, blk * 128:(blk + 1) * 128],
                        rhs=wcomb_sb[:, wblk * 128:(wblk + 1) * 128],
                        start=st, stop=not st))
                for x, y in zip(och, och[1:]):
                    _dep(y, x)
                out_sb = evp.tile([128, 512], bf16, tag="out_sb")
                nc.vector.tensor_copy(
                    out_sb[:].rearrange("p (d c) -> p c d", c=4),
                    o_ps[:].rearrange("p (c d) -> p c d", c=4))
                eng = nc.sync if t == NT - 1 else nc.scalar
                eng.dma_start(nout[t * 128:(t + 1) * 128, :], out_sb[:])

                # -- interleaved sc tiles (species linear, flipped so node
                # runs sit on the free dim where any split offset is legal)
                for gt in sc_of_t[t]:
                    ps_sc = pso.tile([128, 512], f32, tag="ops", space="PSUM")
                    chain = []
                    for (r0, nr, sp) in runs_of_tile[gt]:
                        for comp in range(4):
                            wsc = (wsc0_sb if comp == 0 else wsc1_sb)[
                                :, sp * 128:(sp + 1) * 128]
                            rhs = nfc[comp][:, gt * 128 + r0:
                                            gt * 128 + r0 + nr]
                            chain.append(nc.tensor.matmul(
                                ps_sc[:, comp * 128 + r0:
                                      comp * 128 + r0 + nr],
                                lhsT=wsc, rhs=rhs, start=True, stop=True))
                    for x, y in zip(chain, chain[1:]):
                        _dep(y, x)
                    sc_sb = sa2.tile([128, 128, 4], bf16, tag="sc_sb")
                    src = ps_sc[:].rearrange("p (m n) -> p n m", m=4)
                    if gt % 2 == 0:
                        nc.vector.tensor_copy(sc_sb[:], src)
                    else:
                        nc.scalar.copy(sc_sb[:], src)
                    nc.gpsimd.dma_start(
                        sc_out[gt * 128:(gt + 1) * 128, :],
                        sc_sb[:].rearrange("p n m -> p (n m)"))

    nc.compile()
    return nc


_IN_CHILD = False


def _child_entry(q, inputs):
    """Fresh-process fallback: the PJRT/axon runtime occasionally fails a
    run and stays wedged for the process; a clean process recovers."""
    global _IN_CHILD
    _IN_CHILD = True
    try:
        q.put(("ok", kernel(**inputs)))
    except BaseException as e:  # noqa: BLE001
        q.put(("err", repr(e)))


def kernel(**inputs):
    meta, per_core, shared, unshard = _host_prep(**inputs)
    key = (meta["NT"], meta["NCH"], meta["NSLOT"], meta["runs_of_tile"])
    if key not in _CACHE:
        _CACHE[key] = _build(meta)
    nc = _CACHE[key]
    in_maps = [dict(pc, **shared) for pc in per_core]
    res = None
    err = None
    for attempt in range(2):
        try:
            res = run_bass_kernel_spmd(nc, in_maps,
                                       core_ids=list(range(NCORES)))
            break
        except Exception as e:
            err = e
            try:
                import jax
                jax.clear_caches()
                import jax.extend.backend as _jeb
                _jeb.clear_backends()
            except Exception:
                pass
    if res is None:
        if _IN_CHILD:
            raise err
        import multiprocessing as mp
        ctx = mp.get_context("spawn")
        for attempt in range(3):
            q = ctx.Queue()
            p = ctx.Process(target=_child_entry, args=(q, inputs))
            p.start()
            try:
                status, payload = q.get(timeout=1800)
            except Exception:
                status, payload = "err", "child timeout"
            p.join(timeout=60)
            if p.is_alive():
                p.terminate()
            if status == "ok":
                return payload
        raise err

    NT, NSLOT = unshard["NT"], unshard["NSLOT"]
    core_of, win_of, slot_of = (unshard["core_of"], unshard["win_of"],
                                unshard["slot_of"])
    node_of_scslot = unshard["node_of_scslot"]
    node_out = np.zeros((N, 128, 4), np.float32)
    sc = np.zeros((N, 128, 4), np.float32)
    rows_all = win_of * W + slot_of          # per-core row in nout
    for k in range(NCORES):
        nk = np.nonzero(core_of == k)[0]
        no_k = np.asarray(res.results[k]["nout"], dtype=np.float32)
        node_out[nk] = no_k[rows_all[nk]].reshape(-1, 128, 4)
        valid = node_of_scslot[k] >= 0
        scT = np.asarray(res.results[k]["sc_out"], dtype=np.float32)
        scT = (scT.reshape(NSLOT // 128, 128, 128, 4)
               .transpose(1, 0, 2, 3).reshape(128, NSLOT, 4))
        sc[node_of_scslot[k][valid]] = scT[:, valid, :].transpose(1, 0, 2)
    return node_out, sc
